# revision 62
# baseline (speedup 1.0000x reference)
"""GAT-style attention (gnn_message_passing) Trainium2 kernel, 8-core row-parallel.

Math (algebraically identical to the reference masked-softmax attention):
  E = relu(h @ P)                 [N,3]
  W' = max(exp(E - 4ln2), 1/16)   (= exp(relu(E))/16, fp16-safe range)
  denom'[i,k] = sum_j A[i,j] W'[j,k]   (k=3 slot sums ones -> rowsum[i])
  R'[i,k] = rowsum[i] / denom'[i,k]
  ct[j,i]  = sum_k W'[j,k] R'[i,k] = rowsum[i] * C[i,j]
  out[i,:] = sum_j A[i,j] ct[j,i] h[j,:]

Two SPMD programs (collectives unavailable on this runtime path; the tiny
[4096,3] W matrix crosses cores via a host gather between programs):
  P1 (per core): W'-shard [512,3] from host-transposed h-shard. The E matmuls
      use h.T as the *stationary* operand so each streams only 3 columns.
  host: concat the 8 W'-shards; pack W'.T, W'|ones; cast A-shard.T to fp8
      (binary, exact) and h to fp16  (pure data movement / layout).
  P2 (per core): split into two i-half sweeps. A.T arrives as two fp8
      column-halves in [p, jc, i] packed order (full DMA bandwidth); half 0's
      denominators (at8-stationary matmuls, [128,4] outputs in one PSUM
      bank), R' (rowsum folded in, so no final scale) and its masked-softmax
      sweep (C.T on PE, mask-multiply on DVE, (A*C).T @ h on PE) start as
      soon as the first half lands, while h and the second A.T half stream
      behind. Half 1's denominators + R' are computed mid-sweep-0; its sweep
      follows back-to-back, hiding half 0's output stores. PE warm-up
      matmuls during the initial load defeat the clock-ramp penalty.
"""

import numpy as np
import ml_dtypes

import concourse.bass as bass
import concourse.mybir as mybir
import concourse.tile as tile
from concourse import bacc
from concourse import bass_utils

N = 4096
D = 512
H = 3
NCORES = 8
SH = N // NCORES          # 512 output rows per core
JC = N // 128             # 32 j-chunks
IC = SH // 128            # 4 i-chunks
DC = D // 128             # 4 d-chunks
F8 = mybir.dt.float8e4
F16 = mybir.dt.float16
F32 = mybir.dt.float32
LN2x4 = float(4.0 * np.log(2.0))   # W scaled by 2^-4 to stay in fp16 range
NP_F8 = ml_dtypes.float8_e4m3


def _body1(tc, hst_in, p_in, w_out):
    """P1: W'-shard [SH,3] from hst [128, IC*DC*128] (h-shard.T, jc-major:
    hst[:, jc, dc, :] = h.T d-chunk dc for j-chunk jc), loaded in 2 pieces.
    The E matmuls use hst as the stationary operand (3-column streams)."""
    nc = tc.nc
    with (
        tc.tile_pool(name="sb1", bufs=1) as sb,
        tc.tile_pool(name="ps1", bufs=1, space="PSUM") as ps,
    ):
        hst = sb.tile([128, IC * DC * 128], F16, tag="hst")
        p16 = sb.tile([128, DC * H], F16, tag="p16")
        wsE = sb.tile([128, IC * H], F16, tag="wsE")
        ebias = sb.tile([128, 1], F32, tag="ebias")
        nc.vector.memset(ebias[:], -LN2x4)
        hst_v = hst[:].rearrange("p (g x) -> g p x", g=2)
        hin_v = hst_in.rearrange("p (g x) -> g p x", g=2)
        for g in range(2):
            nc.sync.dma_start(out=hst_v[g], in_=hin_v[g])
        nc.gpsimd.dma_start(out=p16[:], in_=p_in)

        # one PSUM tile spanning 4 banks: E group per jc, single exp at the end
        psE = ps.tile([128, IC * 512], F32, tag="psE", name="psE")
        for jc in range(IC):
            for dc in range(DC):
                nc.tensor.matmul(
                    psE[:, jc * 512: jc * 512 + H],
                    hst[:, (jc * DC + dc) * 128: (jc * DC + dc + 1) * 128],
                    p16[:, dc * H:(dc + 1) * H],
                    start=(dc == 0),
                    stop=(dc == DC - 1),
                )
        nc.scalar.activation(
            wsE[:].rearrange("p (jc k) -> p jc k", k=H),
            psE[:].rearrange("p (jc x) -> p jc x", x=512)[:, :, 0:H],
            mybir.ActivationFunctionType.Exp,
            bias=ebias[:], scale=1.0,
        )
        nc.vector.tensor_scalar_max(wsE[:], wsE[:], 0.0625)
        nc.sync.dma_start(out=w_out, in_=wsE[:])


def _body2(tc, a8a_in, a8b_in, h_in, wt_in, w4_in, id_in, out):
    """P2: the heavy pipeline, split into two i-half sweeps so the first half
    of the masked-softmax loop starts as soon as the first half of A.T has
    landed. a8a_in/a8b_in are A-shard.T column halves in [p, jc, 256] packed
    order (8 KB contiguous per partition -> full DMA bandwidth); wt_in [3,N] /
    w4_in [128,JC*4] are host layouts of the device-computed (scaled) W'."""
    nc = tc.nc
    mult = mybir.AluOpType.mult
    HW = SH // 2              # 256 i per half

    with (
        tc.tile_pool(name="big", bufs=1) as big,
        tc.tile_pool(name="small", bufs=1) as small,
        tc.tile_pool(name="mtp", bufs=4) as mtp,
        tc.tile_pool(name="osb", bufs=4) as osb,
        tc.tile_pool(name="psa", bufs=3, space="PSUM") as psa,
        tc.tile_pool(name="psd", bufs=1, space="PSUM") as psd,
        tc.tile_pool(name="pso", bufs=1, space="PSUM") as pso,
    ):
        at8 = [
            big.tile([128, JC * HW], F8, tag=f"at8{hf}", name=f"at8{hf}")
            for hf in range(2)
        ]                                                   # A.T col-halves
        h16 = big.tile([128, JC * D], F16, tag="h16")       # h, j on partitions
        wt = small.tile([3, N], F16, tag="wt")              # W'.T
        w4 = small.tile([128, JC * 4], F16, tag="w4")       # W'|ones (j on part)
        id16 = small.tile([128, 128], F16, tag="id16")
        scr = small.tile([128, 512], F16, tag="scr")        # warm-up source
        rN16 = small.tile([128, IC * H], F16, tag="rN16")   # rowsum/denom'
        rT16 = [
            small.tile([3, HW], F16, tag=f"rT16{hf}", name=f"rT16{hf}")
            for hf in range(2)
        ]                                                   # R'.T halves [k, i]

        # ---------------- loads ----------------
        # Order on the HWDGE/sync queue: A.T half 0, three h pieces (so the
        # first sweep never waits h), A.T half 1, rest of h. Small fp16 tiles
        # on the SWDGE/gpsimd queue.
        h_r = h_in.rearrange("(g jc p) d -> g p jc d", p=128, g=8)
        h16_v = h16[:].rearrange("p (g jc d) -> g p jc d", g=8, jc=JC // 8)
        a_ins = [a8a_in, a8b_in]
        PHW = (JC // 2) * HW

        def a_piece(hf, piece):
            nc.sync.dma_start(
                out=at8[hf][:, piece * PHW:(piece + 1) * PHW],
                in_=a_ins[hf][:, piece * PHW:(piece + 1) * PHW],
            )

        a_piece(0, 0)
        nc.sync.dma_start(out=id16[:], in_=id_in)   # needed by R transposes
        a_piece(0, 1)
        for g in range(3):
            nc.sync.dma_start(out=h16_v[g], in_=h_r[g])
        a_piece(1, 0)
        a_piece(1, 1)
        for g in range(3, 8):
            nc.sync.dma_start(out=h16_v[g], in_=h_r[g])
        nc.gpsimd.dma_start(out=w4[:], in_=w4_in)
        nc.gpsimd.dma_start(out=wt[:], in_=wt_in)

        nc.vector.memset(scr[:], 0.0)
        # warm the ACT table (LoadActFuncSet) off the critical path
        actw = small.tile([1, 2], F16, tag="actw")
        nc.scalar.copy(actw[:], scr[0:1, 0:2])
        n_warm = 0

        def warm(n):
            nonlocal n_warm
            for _ in range(n):
                pw = psa.tile([128, 512], F32, tag="scr", name=f"warm{n_warm}")
                nc.tensor.matmul(
                    pw[:], scr[:, 0:128], scr[:], start=True, stop=True
                )
                n_warm += 1

        # psD2[p_i, ic*4+k] = sum_j A[i,j] W'[j,k];  k=3 gives rowsum.
        # One accumulation group per half (two ic regions share the group).
        psD2 = psd.tile([128, IC * 4], F32, tag="psd", name="psD2")
        psD2_v = psD2[:].rearrange("p (ic s) -> p ic s", s=4)
        rN = small.tile([128, IC * H], F32, tag="rN")

        def denoms(hf):
            for i2 in range(2):
                ic = hf * 2 + i2
                for jc in range(JC):
                    nc.tensor.matmul(
                        psD2[:, ic * 4:(ic + 1) * 4],
                        at8[hf][:, jc * HW + i2 * 128: jc * HW + i2 * 128 + 128],
                        w4[:, jc * 4:(jc + 1) * 4],
                        start=(i2 == 0 and jc == 0),
                        stop=(i2 == 1 and jc == JC - 1),
                    )

        def r_chain(hf):
            # R' = rowsum * 1/denom' for this half, transposed to [k, i]
            nc.vector.reciprocal(
                rN[:].rearrange("p (ic k) -> p ic k", k=H)[:, 2 * hf:2 * hf + 2],
                psD2_v[:, 2 * hf:2 * hf + 2, 0:H],
            )
            psRT = psa.tile([3, HW], F16, tag="scr", name=f"psRT{hf}")
            for i2 in range(2):
                ic = hf * 2 + i2
                nc.vector.tensor_scalar(
                    rN16[:, ic * H:(ic + 1) * H], rN[:, ic * H:(ic + 1) * H],
                    psD2[:, ic * 4 + 3: ic * 4 + 4], None, op0=mult,
                )
                nc.tensor.transpose(
                    psRT[:, i2 * 128:(i2 + 1) * 128],
                    rN16[:, ic * H:(ic + 1) * H],
                    id16[:],
                )
            nc.vector.tensor_copy(rT16[hf][:], psRT[:])

        psO = [
            pso.tile([128, D], F32, tag=f"psO{ic}", name=f"psO{ic}")
            for ic in range(IC)
        ]
        cts = {}

        def emit_ct(hf, j):
            cts[hf, j] = psa.tile([128, HW], F32, tag="scr", name=f"ct{hf}_{j}")
            nc.tensor.matmul(
                cts[hf, j][:],
                wt[0:3, j * 128:(j + 1) * 128],
                rT16[hf][:],
                start=True,
                stop=True,
                tile_position=(0, 0),
            )

        def store(ic, engine, queue):
            ot = osb.tile([128, D], F16, tag="ot", name=f"ot{ic}")
            if engine == "act":
                nc.scalar.copy(ot[:], psO[ic][:])
            else:
                nc.vector.tensor_copy(ot[:], psO[ic][:])
            queue.dma_start(out=out_r[ic], in_=ot[:])

        out_r = out.rearrange("(ic p) d -> ic p d", p=128)

        def sweep(hf):
            emit_ct(hf, 0)
            emit_ct(hf, 1)
            for jc in range(JC):
                mt = mtp.tile([128, HW], F16, tag="mt", name=f"mt{hf}_{jc}")
                nc.vector.tensor_tensor(
                    mt[:], at8[hf][:, jc * HW:(jc + 1) * HW], cts[hf, jc][:],
                    op=mult,
                )
                if jc + 2 < JC:
                    emit_ct(hf, jc + 2)
                if hf == 0 and jc == 20:
                    # second half's denominators + R while sweep 0 finishes
                    denoms(1)
                    r_chain(1)
                for i2 in range(2):
                    nc.tensor.matmul(
                        psO[hf * 2 + i2][:],
                        mt[:, i2 * 128:(i2 + 1) * 128],
                        h16[:, jc * D:(jc + 1) * D],
                        start=(jc == 0),
                        stop=(jc == JC - 1),
                    )

        warm(12)
        denoms(0)
        r_chain(0)
        sweep(0)
        # first-half stores on ACT + SWDGE: fully hidden under sweep 1
        store(0, "act", nc.gpsimd)
        store(1, "act", nc.gpsimd)
        sweep(1)
        store(2, "dve", nc.sync)
        store(3, "act", nc.sync)


_CACHE = {}


def _build1():
    if "p1" in _CACHE:
        return _CACHE["p1"]
    nc = bacc.Bacc("TRN2", target_bir_lowering=False, debug=False,
                   num_devices=NCORES)
    hst_in = nc.dram_tensor("hst_in", [128, IC * DC * 128], F16,
                            kind="ExternalInput").ap()
    p_in = nc.dram_tensor("p_in", [128, DC * H], F16, kind="ExternalInput").ap()
    w_out = nc.dram_tensor("w_out", [128, IC * H], F16,
                           kind="ExternalOutput").ap()
    with tile.TileContext(nc) as tc:
        _body1(tc, hst_in, p_in, w_out)
    nc.compile()
    _CACHE["p1"] = nc
    return nc


def _build2():
    if "p2" in _CACHE:
        return _CACHE["p2"]
    nc = bacc.Bacc("TRN2", target_bir_lowering=False, debug=False,
                   num_devices=NCORES)
    a8a_in = nc.dram_tensor("a8a_in", [128, JC * SH // 2], F8,
                            kind="ExternalInput").ap()
    a8b_in = nc.dram_tensor("a8b_in", [128, JC * SH // 2], F8,
                            kind="ExternalInput").ap()
    h_in = nc.dram_tensor("h_in", [N, D], F16, kind="ExternalInput").ap()
    wt_in = nc.dram_tensor("wt_in", [3, N], F16, kind="ExternalInput").ap()
    w4_in = nc.dram_tensor("w4_in", [128, JC * 4], F16,
                           kind="ExternalInput").ap()
    id_in = nc.dram_tensor("id_in", [128, 128], F16, kind="ExternalInput").ap()
    out = nc.dram_tensor("out", [SH, D], F16, kind="ExternalOutput").ap()
    with tile.TileContext(nc) as tc:
        _body2(tc, a8a_in, a8b_in, h_in, wt_in, w4_in, id_in, out)
    nc.compile()
    _CACHE["p2"] = nc
    return nc


def kernel(graph_info, h, P, _trace=False, _results_out=None):
    graph_info = np.ascontiguousarray(graph_info, dtype=np.float32)
    h = np.ascontiguousarray(h, dtype=np.float32)
    P = np.ascontiguousarray(P, dtype=np.float32)
    nc1 = _build1()
    nc2 = _build2()

    # host-side shard/layout prep (pure data movement + dtype casts)
    h16_full = h.astype(np.float16)
    p16_host = np.ascontiguousarray(
        P.astype(np.float16).reshape(DC, 128, H).transpose(1, 0, 2)
    ).reshape(128, DC * H)
    in1 = []
    for c in range(NCORES):
        hsT = h16_full[c * SH:(c + 1) * SH, :].T  # [D, SH]
        hst_host = np.ascontiguousarray(
            hsT.reshape(DC, 128, IC, 128).transpose(1, 2, 0, 3)
        ).reshape(128, IC * DC * 128)
        in1.append({"hst_in": hst_host, "p_in": p16_host})
    res1 = bass_utils.run_bass_kernel_spmd(
        nc1, in1, core_ids=list(range(NCORES)), trace=_trace
    )
    w_full = np.concatenate(
        [
            res1.results[c]["w_out"]
            .reshape(128, IC, H).transpose(1, 0, 2).reshape(SH, H)
            for c in range(NCORES)
        ],
        axis=0,
    )  # [N, 3] fp16, scaled by 2^-4

    wt_host = np.ascontiguousarray(w_full.T)  # [3, N]
    w4_host = np.ascontiguousarray(
        np.concatenate(
            [w_full.reshape(JC, 128, H).transpose(1, 0, 2),
             np.ones((128, JC, 1), np.float16)],
            axis=2,
        ).reshape(128, JC * 4)
    )
    id_host = np.eye(128, dtype=np.float16)

    in2 = []
    for c in range(NCORES):
        at = np.ascontiguousarray(
            graph_info[c * SH:(c + 1) * SH, :].T
        ).astype(NP_F8)                      # [N(j), SH(i)]
        x = at.reshape(JC, 128, SH).transpose(1, 0, 2)   # [p, jc, i]
        in2.append({
            "a8a_in": np.ascontiguousarray(x[:, :, 0:SH // 2]).reshape(128, -1),
            "a8b_in": np.ascontiguousarray(x[:, :, SH // 2:]).reshape(128, -1),
            "h_in": h16_full,
            "wt_in": wt_host,
            "w4_in": w4_host,
            "id_in": id_host,
        })
    res2 = bass_utils.run_bass_kernel_spmd(
        nc2, in2, core_ids=list(range(NCORES)), trace=_trace
    )
    if _results_out is not None:
        _results_out.extend([res1, res2])
    return np.concatenate(
        [res2.results[c]["out"].astype(np.float32) for c in range(NCORES)],
        axis=0,
    )


# revision 66
# speedup vs baseline: 1.0179x; 1.0179x over previous
"""GAT-style attention (gnn_message_passing) Trainium2 kernel, 8-core row-parallel.

Math (algebraically identical to the reference masked-softmax attention):
  E = relu(h @ P)                 [N,3]
  W' = max(exp(E - 4ln2), 1/16)   (= exp(relu(E))/16, fp16-safe range)
  denom'[i,k] = sum_j A[i,j] W'[j,k]   (k=3 slot sums ones -> rowsum[i])
  R'[i,k] = rowsum[i] / denom'[i,k]
  ct[j,i]  = sum_k W'[j,k] R'[i,k] = rowsum[i] * C[i,j]
  out[i,:] = sum_j A[i,j] ct[j,i] h[j,:]

Two SPMD programs (collectives unavailable on this runtime path; the tiny
[4096,3] W matrix crosses cores via a host gather between programs):
  P1 (per core): W'-shard [512,3] from host-transposed h-shard. The E matmuls
      use h.T as the *stationary* operand so each streams only 3 columns.
  host: concat the 8 W'-shards; pack W'.T, W'|ones; cast A-shard.T to fp8
      (binary, exact) and h to fp16  (pure data movement / layout).
  P2 (per core): split into two i-half sweeps. A.T arrives as two fp8
      column-halves in [p, jc, i] packed order (full DMA bandwidth); half 0's
      denominators (at8-stationary matmuls, [128,4] outputs in one PSUM
      bank), R' (rowsum folded in, so no final scale) and its masked-softmax
      sweep (C.T on PE, mask-multiply on DVE, (A*C).T @ h on PE) start as
      soon as the first half lands, while h and the second A.T half stream
      behind. Half 1's denominators + R' are computed mid-sweep-0; its sweep
      follows back-to-back, hiding half 0's output stores. PE warm-up
      matmuls during the initial load defeat the clock-ramp penalty.
"""

import numpy as np
import ml_dtypes

import concourse.bass as bass
import concourse.mybir as mybir
import concourse.tile as tile
from concourse import bacc
from concourse import bass_utils

N = 4096
D = 512
H = 3
NCORES = 8
SH = N // NCORES          # 512 output rows per core
JC = N // 128             # 32 j-chunks
IC = SH // 128            # 4 i-chunks
DC = D // 128             # 4 d-chunks
F8 = mybir.dt.float8e4
F16 = mybir.dt.float16
F32 = mybir.dt.float32
LN2x4 = float(4.0 * np.log(2.0))   # W scaled by 2^-4 to stay in fp16 range
NP_F8 = ml_dtypes.float8_e4m3


def _body1(tc, hst_in, p_in, w_out):
    """P1: W'-shard [SH,3] from hst [128, IC*DC*128] (h-shard.T, jc-major:
    hst[:, jc, dc, :] = h.T d-chunk dc for j-chunk jc), loaded in 2 pieces.
    The E matmuls use hst as the stationary operand (3-column streams)."""
    nc = tc.nc
    with (
        tc.tile_pool(name="sb1", bufs=1) as sb,
        tc.tile_pool(name="ps1", bufs=1, space="PSUM") as ps,
    ):
        hst = sb.tile([128, IC * DC * 128], F16, tag="hst")
        p16 = sb.tile([128, DC * H], F16, tag="p16")
        wsE = sb.tile([128, IC * H], F16, tag="wsE")
        ebias = sb.tile([128, 1], F32, tag="ebias")
        nc.vector.memset(ebias[:], -LN2x4)
        hst_v = hst[:].rearrange("p (g x) -> g p x", g=2)
        hin_v = hst_in.rearrange("p (g x) -> g p x", g=2)
        for g in range(2):
            nc.sync.dma_start(out=hst_v[g], in_=hin_v[g])
        nc.gpsimd.dma_start(out=p16[:], in_=p_in)

        # one PSUM tile spanning 4 banks: E group per jc, single exp at the end
        psE = ps.tile([128, IC * 512], F32, tag="psE", name="psE")
        for jc in range(IC):
            for dc in range(DC):
                nc.tensor.matmul(
                    psE[:, jc * 512: jc * 512 + H],
                    hst[:, (jc * DC + dc) * 128: (jc * DC + dc + 1) * 128],
                    p16[:, dc * H:(dc + 1) * H],
                    start=(dc == 0),
                    stop=(dc == DC - 1),
                )
        nc.scalar.activation(
            wsE[:].rearrange("p (jc k) -> p jc k", k=H),
            psE[:].rearrange("p (jc x) -> p jc x", x=512)[:, :, 0:H],
            mybir.ActivationFunctionType.Exp,
            bias=ebias[:], scale=1.0,
        )
        nc.vector.tensor_scalar_max(wsE[:], wsE[:], 0.0625)
        # SWDGE: descriptor generation pre-runs on the idle Pool engine
        # during the hst load, so only DGE delay + transfer remain at the end
        nc.gpsimd.dma_start(out=w_out, in_=wsE[:])


def _body2(tc, a8a_in, a8b_in, h_in, wt_in, w4_in, id_in, out):
    """P2: the heavy pipeline, split into two i-half sweeps so the first half
    of the masked-softmax loop starts as soon as the first half of A.T has
    landed. a8a_in/a8b_in are A-shard.T column halves in [p, jc, 256] packed
    order (8 KB contiguous per partition -> full DMA bandwidth); wt_in [3,N] /
    w4_in [128,JC*4] are host layouts of the device-computed (scaled) W'."""
    nc = tc.nc
    mult = mybir.AluOpType.mult
    HW = SH // 2              # 256 i per half

    with (
        tc.tile_pool(name="big", bufs=1) as big,
        tc.tile_pool(name="small", bufs=1) as small,
        tc.tile_pool(name="mtp", bufs=4) as mtp,
        tc.tile_pool(name="osb", bufs=4) as osb,
        tc.tile_pool(name="psa", bufs=3, space="PSUM") as psa,
        tc.tile_pool(name="psd", bufs=1, space="PSUM") as psd,
        tc.tile_pool(name="pso", bufs=1, space="PSUM") as pso,
    ):
        at8 = [
            big.tile([128, JC * HW], F8, tag=f"at8{hf}", name=f"at8{hf}")
            for hf in range(2)
        ]                                                   # A.T col-halves
        h16 = big.tile([128, JC * D], F16, tag="h16")       # h, j on partitions
        wt = small.tile([3, N], F16, tag="wt")              # W'.T
        w4 = small.tile([128, JC * 4], F16, tag="w4")       # W'|ones (j on part)
        id16 = small.tile([128, 128], F16, tag="id16")
        scr = small.tile([128, 512], F16, tag="scr")        # warm-up source
        rN16 = small.tile([128, IC * H], F16, tag="rN16")   # rowsum/denom'
        rT16 = [
            small.tile([3, HW], F16, tag=f"rT16{hf}", name=f"rT16{hf}")
            for hf in range(2)
        ]                                                   # R'.T halves [k, i]

        # ---------------- loads ----------------
        # Order on the HWDGE/sync queue: A.T half 0, three h pieces (so the
        # first sweep never waits h), A.T half 1, rest of h. Small fp16 tiles
        # on the SWDGE/gpsimd queue.
        h_r = h_in.rearrange("(g jc p) d -> g p jc d", p=128, g=8)
        h16_v = h16[:].rearrange("p (g jc d) -> g p jc d", g=8, jc=JC // 8)
        a_ins = [a8a_in, a8b_in]
        PHW = (JC // 2) * HW

        def a_piece(hf, piece):
            nc.sync.dma_start(
                out=at8[hf][:, piece * PHW:(piece + 1) * PHW],
                in_=a_ins[hf][:, piece * PHW:(piece + 1) * PHW],
            )

        a_piece(0, 0)
        nc.sync.dma_start(out=id16[:], in_=id_in)   # needed by R transposes
        a_piece(0, 1)
        for g in range(4):
            nc.sync.dma_start(out=h16_v[g], in_=h_r[g])
        a_piece(1, 0)
        a_piece(1, 1)
        for g in range(4, 8):
            nc.sync.dma_start(out=h16_v[g], in_=h_r[g])
        nc.gpsimd.dma_start(out=w4[:], in_=w4_in)
        nc.gpsimd.dma_start(out=wt[:], in_=wt_in)

        nc.vector.memset(scr[:], 0.0)
        # warm the ACT table (LoadActFuncSet) off the critical path
        actw = small.tile([1, 2], F16, tag="actw")
        nc.scalar.copy(actw[:], scr[0:1, 0:2])
        n_warm = 0

        def warm(n):
            nonlocal n_warm
            for _ in range(n):
                pw = psa.tile([128, 512], F32, tag="scr", name=f"warm{n_warm}")
                nc.tensor.matmul(
                    pw[:], scr[:, 0:128], scr[:], start=True, stop=True
                )
                n_warm += 1

        # psD2[p_i, ic*4+k] = sum_j A[i,j] W'[j,k];  k=3 gives rowsum.
        # One accumulation group per half (two ic regions share the group).
        psD2 = psd.tile([128, IC * 4], F32, tag="psd", name="psD2")
        psD2_v = psD2[:].rearrange("p (ic s) -> p ic s", s=4)
        rN = small.tile([128, IC * H], F32, tag="rN")

        def denoms(hf):
            for i2 in range(2):
                ic = hf * 2 + i2
                for jc in range(JC):
                    nc.tensor.matmul(
                        psD2[:, ic * 4:(ic + 1) * 4],
                        at8[hf][:, jc * HW + i2 * 128: jc * HW + i2 * 128 + 128],
                        w4[:, jc * 4:(jc + 1) * 4],
                        start=(i2 == 0 and jc == 0),
                        stop=(i2 == 1 and jc == JC - 1),
                    )

        def r_chain(hf):
            # R' = rowsum * 1/denom' for this half, transposed to [k, i]
            nc.vector.reciprocal(
                rN[:].rearrange("p (ic k) -> p ic k", k=H)[:, 2 * hf:2 * hf + 2],
                psD2_v[:, 2 * hf:2 * hf + 2, 0:H],
            )
            psRT = psa.tile([3, HW], F16, tag="scr", name=f"psRT{hf}")
            for i2 in range(2):
                ic = hf * 2 + i2
                nc.vector.tensor_scalar(
                    rN16[:, ic * H:(ic + 1) * H], rN[:, ic * H:(ic + 1) * H],
                    psD2[:, ic * 4 + 3: ic * 4 + 4], None, op0=mult,
                )
                nc.tensor.transpose(
                    psRT[:, i2 * 128:(i2 + 1) * 128],
                    rN16[:, ic * H:(ic + 1) * H],
                    id16[:],
                )
            nc.vector.tensor_copy(rT16[hf][:], psRT[:])

        psO = [
            pso.tile([128, D], F32, tag=f"psO{ic}", name=f"psO{ic}")
            for ic in range(IC)
        ]
        cts = {}

        def emit_ct(hf, j):
            cts[hf, j] = psa.tile([128, HW], F32, tag="scr", name=f"ct{hf}_{j}")
            nc.tensor.matmul(
                cts[hf, j][:],
                wt[0:3, j * 128:(j + 1) * 128],
                rT16[hf][:],
                start=True,
                stop=True,
                tile_position=(0, 0),
            )

        def store(ic, engine, queue):
            ot = osb.tile([128, D], F16, tag="ot", name=f"ot{ic}")
            if engine == "act":
                nc.scalar.copy(ot[:], psO[ic][:])
            else:
                nc.vector.tensor_copy(ot[:], psO[ic][:])
            queue.dma_start(out=out_r[ic], in_=ot[:])

        out_r = out.rearrange("(ic p) d -> ic p d", p=128)

        def sweep(hf):
            if hf == 0:
                emit_ct(hf, 0)
                emit_ct(hf, 1)
            for jc in range(JC):
                mt = mtp.tile([128, HW], F16, tag="mt", name=f"mt{hf}_{jc}")
                nc.vector.tensor_tensor(
                    mt[:], at8[hf][:, jc * HW:(jc + 1) * HW], cts[hf, jc][:],
                    op=mult,
                )
                if jc + 2 < JC:
                    emit_ct(hf, jc + 2)
                if hf == 0 and jc == 20:
                    # second half's denominators + R while sweep 0 finishes
                    denoms(1)
                    r_chain(1)
                if hf == 0 and jc in (29, 30):
                    # pre-emit sweep 1's lead cts into buffers already freed
                    emit_ct(1, jc - 29)
                for i2 in range(2):
                    nc.tensor.matmul(
                        psO[hf * 2 + i2][:],
                        mt[:, i2 * 128:(i2 + 1) * 128],
                        h16[:, jc * D:(jc + 1) * D],
                        start=(jc == 0),
                        stop=(jc == JC - 1),
                    )

        warm(12)
        denoms(0)
        r_chain(0)
        sweep(0)
        # first-half stores on ACT + SWDGE: fully hidden under sweep 1
        store(0, "act", nc.gpsimd)
        store(1, "act", nc.gpsimd)
        sweep(1)
        store(2, "dve", nc.sync)
        store(3, "act", nc.sync)


_CACHE = {}


def _build1():
    if "p1" in _CACHE:
        return _CACHE["p1"]
    nc = bacc.Bacc("TRN2", target_bir_lowering=False, debug=False,
                   num_devices=NCORES)
    hst_in = nc.dram_tensor("hst_in", [128, IC * DC * 128], F16,
                            kind="ExternalInput").ap()
    p_in = nc.dram_tensor("p_in", [128, DC * H], F16, kind="ExternalInput").ap()
    w_out = nc.dram_tensor("w_out", [128, IC * H], F16,
                           kind="ExternalOutput").ap()
    with tile.TileContext(nc) as tc:
        _body1(tc, hst_in, p_in, w_out)
    nc.compile()
    _CACHE["p1"] = nc
    return nc


def _build2():
    if "p2" in _CACHE:
        return _CACHE["p2"]
    nc = bacc.Bacc("TRN2", target_bir_lowering=False, debug=False,
                   num_devices=NCORES)
    a8a_in = nc.dram_tensor("a8a_in", [128, JC * SH // 2], F8,
                            kind="ExternalInput").ap()
    a8b_in = nc.dram_tensor("a8b_in", [128, JC * SH // 2], F8,
                            kind="ExternalInput").ap()
    h_in = nc.dram_tensor("h_in", [N, D], F16, kind="ExternalInput").ap()
    wt_in = nc.dram_tensor("wt_in", [3, N], F16, kind="ExternalInput").ap()
    w4_in = nc.dram_tensor("w4_in", [128, JC * 4], F16,
                           kind="ExternalInput").ap()
    id_in = nc.dram_tensor("id_in", [128, 128], F16, kind="ExternalInput").ap()
    out = nc.dram_tensor("out", [SH, D], F16, kind="ExternalOutput").ap()
    with tile.TileContext(nc) as tc:
        _body2(tc, a8a_in, a8b_in, h_in, wt_in, w4_in, id_in, out)
    nc.compile()
    _CACHE["p2"] = nc
    return nc


def kernel(graph_info, h, P, _trace=False, _results_out=None):
    graph_info = np.ascontiguousarray(graph_info, dtype=np.float32)
    h = np.ascontiguousarray(h, dtype=np.float32)
    P = np.ascontiguousarray(P, dtype=np.float32)
    nc1 = _build1()
    nc2 = _build2()

    # host-side shard/layout prep (pure data movement + dtype casts)
    h16_full = h.astype(np.float16)
    p16_host = np.ascontiguousarray(
        P.astype(np.float16).reshape(DC, 128, H).transpose(1, 0, 2)
    ).reshape(128, DC * H)
    in1 = []
    for c in range(NCORES):
        hsT = h16_full[c * SH:(c + 1) * SH, :].T  # [D, SH]
        hst_host = np.ascontiguousarray(
            hsT.reshape(DC, 128, IC, 128).transpose(1, 2, 0, 3)
        ).reshape(128, IC * DC * 128)
        in1.append({"hst_in": hst_host, "p_in": p16_host})
    res1 = bass_utils.run_bass_kernel_spmd(
        nc1, in1, core_ids=list(range(NCORES)), trace=_trace
    )
    w_full = np.concatenate(
        [
            res1.results[c]["w_out"]
            .reshape(128, IC, H).transpose(1, 0, 2).reshape(SH, H)
            for c in range(NCORES)
        ],
        axis=0,
    )  # [N, 3] fp16, scaled by 2^-4

    wt_host = np.ascontiguousarray(w_full.T)  # [3, N]
    w4_host = np.ascontiguousarray(
        np.concatenate(
            [w_full.reshape(JC, 128, H).transpose(1, 0, 2),
             np.ones((128, JC, 1), np.float16)],
            axis=2,
        ).reshape(128, JC * 4)
    )
    id_host = np.eye(128, dtype=np.float16)

    in2 = []
    for c in range(NCORES):
        at = np.ascontiguousarray(
            graph_info[c * SH:(c + 1) * SH, :].T
        ).astype(NP_F8)                      # [N(j), SH(i)]
        x = at.reshape(JC, 128, SH).transpose(1, 0, 2)   # [p, jc, i]
        in2.append({
            "a8a_in": np.ascontiguousarray(x[:, :, 0:SH // 2]).reshape(128, -1),
            "a8b_in": np.ascontiguousarray(x[:, :, SH // 2:]).reshape(128, -1),
            "h_in": h16_full,
            "wt_in": wt_host,
            "w4_in": w4_host,
            "id_in": id_host,
        })
    res2 = bass_utils.run_bass_kernel_spmd(
        nc2, in2, core_ids=list(range(NCORES)), trace=_trace
    )
    if _results_out is not None:
        _results_out.extend([res1, res2])
    return np.concatenate(
        [res2.results[c]["out"].astype(np.float32) for c in range(NCORES)],
        axis=0,
    )
